# revision 1
# baseline (speedup 1.0000x reference)
"""AttentionBlock kernel for 8 Trainium2 NeuronCores.

Problem (hardcoded): x [4, 2048, 1024] f32; Wq/Wk/Wv/Wfc [1024, 1024]; biases [1024].
    q = x@Wq.T+bq; k = x@Wk.T+bk; v = x@Wv.T+bv
    out = softmax(q k^T / sqrt(1024)) v;  y = out@Wfc.T+bfc + x

Sharding: core i = (b = i//2, h = i%2). Each core computes the full V / scores for
its batch element (duplicated across the 2 cores sharing a batch) and the
attention + fc for its half of the sequence. No collectives (measured ~40us fixed
+ ~7.6us/MB per 2-core AllGather here -- a K/V exchange costs more than it saves).

Key algebraic trick: q k^T = x (Wq^T Wk) x^T, so the host pre-contracts
M = Wq^T @ Wk and the kernel never materializes Q or K:
    G^T = M-blocks^T @ xT           (27us instead of Q-proj 27 + K-proj 55)
    S^T = xT-blocks^T @ G^T         (55us, lhsT streamed straight from x!)
The bias cross-terms are exact: the per-q term and constant cancel in softmax;
the per-k term r2[k] = x_k . (Wk^T bq) is a cheap rank-1 matmul folded into the
exp's per-partition bias.

Per-core plan (all matmuls float32r = full PE rate, ~2e-4 rel err):
  host feeds xT = x[b].T (d-major, rolled so this core's q-half is columns 0:1024)
  plus M, Wv^T, Wfc^T, so every GEMM has its contraction dim on partitions with
  no on-device transposes.
  - G^T [d, q] resident; V [s, e] resident (one xT sweep); r2 column per k-block
  - attention per q-chunk of 512: S^T blocks with xT streamed as lhsT, softmax
    over the partition (k) axis: exp(scale*S + r2) without max-subtract
    (|S|*scale <~ 6 here), denominator via ones-matmul, reciprocal broadcast
    across partitions with a rank-1 PE matmul, U^T = V-block.T @ expS^T
    accumulated in PSUM and normalized on copy-out -> O^T spilled to DRAM
  - fc: y = (O^T-block).T @ Wfc^T + bfc + x
"""

import numpy as np

B, S, DIM = 4, 2048, 1024
P = 128
NCORES = 8
HALF = S // 2          # 1024 q rows per core
DT = DIM // P          # 8 d tiles
ET = DIM // P          # 8 e tiles
SCH = S // 512         # 4 s-chunks for the V sweep
QC = 512               # attention q-chunk
NQ = HALF // QC        # 2 q chunks
KB = S // P            # 16 k blocks
SCALE = 1.0 / float(np.sqrt(DIM))

_CACHE = {}
TIMING_REPEAT = 21


def _build(repeat=1):
    import concourse.mybir as mybir
    import concourse.tile as tile
    from concourse import bacc

    F32 = mybir.dt.float32
    F32R = mybir.dt.float32r
    EXP = mybir.ActivationFunctionType.Exp
    IDENT = mybir.ActivationFunctionType.Identity
    ADD = mybir.AluOpType.add
    MULT = mybir.AluOpType.mult

    nc = bacc.Bacc()

    xt_d = nc.dram_tensor("xt", [DIM, S], F32R, kind="ExternalInput")
    xr_d = nc.dram_tensor("xr", [HALF, DIM], F32, kind="ExternalInput")
    m_d = nc.dram_tensor("m", [DIM, DIM], F32R, kind="ExternalInput")   # Wq^T Wk
    n_d = nc.dram_tensor("n", [DIM, DIM], F32R, kind="ExternalInput")   # Wv^T Wfc^T
    xn_d = nc.dram_tensor("xn", [S, DIM], F32R, kind="ExternalInput")   # x natural, rolled
    r2_d = nc.dram_tensor("r2", [S], F32, kind="ExternalInput")  # scale*x.(Wk^T bq)
    y_d = nc.dram_tensor("y", [HALF, DIM], F32, kind="ExternalOutput")

    xt3 = xt_d[:].rearrange("(dt p) s -> p dt s", p=P)      # [128, 8, 2048]
    m3 = m_d[:].rearrange("(dt p) e -> p dt e", p=P)
    n3 = n_d[:].rearrange("(dt p) e -> p dt e", p=P)
    xn3 = xn_d[:].rearrange("(kb p) d -> p kb d", p=P)      # [128, 16, 1024]

    with tile.TileContext(nc, pool_alloc_mode="stack") as tc:
        cpool = tc.alloc_tile_pool(name="const", bufs=1)
        ones2 = cpool.tile([P, 2], F32R)   # HAM-warmup lhsT (even-N fp32r rule)
        onesk = cpool.tile([P, 1], F32R)   # denominator-row lhsT
        ones_f32 = cpool.tile([P, P], F32)
        nc.vector.memset(ones_f32[:], 1.0)
        nc.vector.tensor_copy(ones2[:], ones_f32[:, 0:2])
        nc.vector.tensor_copy(onesk[:], ones_f32[:, 0:1])
        # warm the ACT LUTs (first use otherwise pays a ~1.4us cold table load)
        warm = cpool.tile([1, 2], F32)
        nc.scalar.activation(warm[0:1, 0:1], ones_f32[0:1, 0:1], IDENT)
        nc.scalar.activation(warm[0:1, 1:2], ones_f32[0:1, 0:1], EXP)
        # warm the PE HAM clock gate during the initial DMA wait: ~4us of dummy
        # matmuls with no input deps so the real work starts at 2.4GHz
        dwarm = cpool.tile([P, 512], F32R)
        nc.vector.memset(ones_f32[:], 1.0)
        nc.vector.tensor_copy(dwarm[:, 0:P], ones_f32[:])
        with tc.tile_pool(name="pwarm", bufs=1, space="PSUM") as pwp:
            pw = pwp.tile([2, 512], F32)
            for i in range(10):
                nc.tensor.matmul(pw[:], ones2[:], dwarm[:],
                                 start=(i == 0), stop=(i == 9))

        for _rep in range(repeat):
            # -------- Phase G: G^T = (Wq^T Wk)-blocks^T @ xT-half (resident) ----
            gpool = tc.alloc_tile_pool(name="gt", bufs=1)
            gt_sb = gpool.tile([P, DT, HALF], F32R, tag="gt")  # [d_p, d_tile, q]
            with tc.tile_pool(name="mq", bufs=1) as mqp, \
                 tc.tile_pool(name="xtq", bufs=2) as xtqp, \
                 tc.tile_pool(name="pq", bufs=3, space="PSUM") as pqp:
                m_sb = mqp.tile([P, DT, DIM], F32R)
                xtq0 = xtqp.tile([P, DT, 512], F32R, tag="xtq")
                # interleave the first loads across all three DMA queues so the
                # first group isn't gated by one queue's serial transfer rate
                engs = (nc.sync, nc.scalar, nc.gpsimd)
                for dt in range(DT):
                    engs[(2 * dt) % 3].dma_start(m_sb[:, dt, :], m3[:, dt, :])
                    engs[(2 * dt + 1) % 3].dma_start(xtq0[:, dt, :], xt3[:, dt, 0:512])
                for qch in range(HALF // 512):
                    if qch == 0:
                        xtq = xtq0
                    else:
                        xtq = xtqp.tile([P, DT, 512], F32R, tag="xtq")
                        nc.sync.dma_start(xtq[:], xt3[:, :, qch * 512:(qch + 1) * 512])
                    for dtile in range(DT):
                        ps = pqp.tile([P, 512], F32, tag="pq")
                        for dt in range(DT):
                            nc.tensor.matmul(
                                ps[:], m_sb[:, dt, dtile * P:(dtile + 1) * P],
                                xtq[:, dt, :],
                                start=(dt == 0), stop=(dt == DT - 1))
                        nc.scalar.activation(
                            gt_sb[:, dtile, qch * 512:(qch + 1) * 512], ps[:], IDENT)

            # ---- Phase X: x natural + N resident (no value projection at all:
            # y = (P~ @ x) @ N by associativity, so the VF sweep is just a load) --
            vpool = tc.alloc_tile_pool(name="xn", bufs=1)
            espool = tc.alloc_tile_pool(name="es", bufs=1)
            xtbpool = tc.alloc_tile_pool(name="xtb", bufs=3)
            npool = tc.alloc_tile_pool(name="n", bufs=1)
            xn_sb = vpool.tile([P, KB, DIM], F32R, tag="xn")  # [k_p, kb, d]
            r2c = vpool.tile([P, KB], F32, tag="r2c")
            n_sb = npool.tile([P, DT, DIM], F32R, tag="n")
            nc.scalar.dma_start(r2c[:], r2_d[:].rearrange("(t p) -> p t", p=P))
            # xn/n aren't needed until the Z/yN matmuls (~40-70us later): keep
            # them off the sync queue (attention x-block stream) AND off the
            # scalar queue (phase G's tail copyouts run there)
            for kb in range(KB):
                nc.gpsimd.dma_start(xn_sb[:, kb, :], xn3[:, kb, :])
            nc.gpsimd.dma_start(n_sb[:], n3[:])

            # ------- Phase A: attention -> y directly (per q-chunk of 512) ------
            # S^T blocks -> exp -> row denominator; Z^T = xn-blocks^T @ es in two
            # 4-bank half-passes; y = Z^T-blocks^T @ N, normalized by 1/denom as
            # a per-partition ACT scale; bfc + Wfc@bv are pre-added into xr.
            with tc.tile_pool(name="zt", bufs=1) as ztp, \
                 tc.tile_pool(name="rec", bufs=2) as recp, \
                 tc.tile_pool(name="xrt", bufs=2) as xrp, \
                 tc.tile_pool(name="ysb", bufs=2) as ysp, \
                 tc.tile_pool(name="ps_s", bufs=2, space="PSUM") as psp, \
                 tc.tile_pool(name="ps_z", bufs=4, space="PSUM") as pzp, \
                 tc.tile_pool(name="ps_d", bufs=1, space="PSUM") as pdp:
                for qc in range(NQ):
                    q0 = qc * QC
                    es = espool.tile([P, KB, QC], F32R, tag="es")  # exp [k_p, kb, q]
                    pdr = pdp.tile([1, QC], F32, tag="ps_d")
                    for kb in range(KB):
                        xtb = xtbpool.tile([P, DT, P], F32R, tag="xtb")
                        nc.sync.dma_start(xtb[:], xt3[:, :, kb * P:(kb + 1) * P])
                        ps = psp.tile([P, QC], F32, tag="ps_s")
                        for dt in range(DT):
                            nc.tensor.matmul(
                                ps[:], xtb[:, dt, :], gt_sb[:, dt, q0:q0 + QC],
                                start=(dt == 0), stop=(dt == DT - 1))
                        nc.scalar.activation(es[:, kb, :], ps[:], EXP,
                                             bias=r2c[:, kb:kb + 1], scale=SCALE)
                        nc.tensor.matmul(pdr[:], onesk[:], es[:, kb, :],
                                         start=(kb == 0), stop=(kb == KB - 1))
                    # reciprocal row -> per-q columns via 4 partition-column DMAs
                    recd = recp.tile([1, QC], F32, tag="recd")
                    nc.vector.reciprocal(recd[:], pdr[:])
                    recq = recp.tile([P, QC // P], F32, tag="recq")
                    for qb in range(QC // P):
                        nc.sync.dma_start(recq[:, qb:qb + 1],
                                          recd[0:1, qb * P:(qb + 1) * P])
                    # Z~^T[d, q] = sum_kb xn-block^T @ es-block, 4 banks per pass
                    zt = ztp.tile([P, DT, QC], F32R, tag="zt")  # [d_p, dt, q]
                    for hf in range(2):
                        pz = [pzp.tile([P, QC], F32, tag="ps_z", name=f"pz{hf}_{i}")
                              for i in range(4)]
                        for kb in range(KB):
                            for i in range(4):
                                dtile = hf * 4 + i
                                nc.tensor.matmul(
                                    pz[i][:],
                                    xn_sb[:, kb, dtile * P:(dtile + 1) * P],
                                    es[:, kb, :],
                                    start=(kb == 0), stop=(kb == KB - 1))
                        for i in range(4):
                            nc.scalar.activation(zt[:, hf * 4 + i, :], pz[i][:], IDENT)
                    # y = Z~^T-blocks^T @ N, scaled by 1/denom; xr carries biases
                    for qb in range(QC // P):
                        q_t = qc * (QC // P) + qb
                        xrt = xrp.tile([P, DIM], F32, tag="xrt")
                        nc.scalar.dma_start(xrt[:], xr_d[q_t * P:(q_t + 1) * P, :])
                        for ec in range(2):
                            py = pzp.tile([P, 512], F32, tag="ps_z",
                                          name=f"py{q_t}_{ec}")
                            for dt in range(DT):
                                nc.tensor.matmul(
                                    py[:], zt[:, dt, qb * P:(qb + 1) * P],
                                    n_sb[:, dt, ec * 512:(ec + 1) * 512],
                                    start=(dt == 0), stop=(dt == DT - 1))
                            ysb = ysp.tile([P, 512], F32, tag="ysb")
                            nc.scalar.activation(ysb[:], py[:], IDENT,
                                                 scale=recq[:, qb:qb + 1])
                            nc.vector.tensor_tensor(
                                ysb[:], ysb[:], xrt[:, ec * 512:(ec + 1) * 512], ADD)
                            nc.gpsimd.dma_start(
                                y_d[q_t * P:(q_t + 1) * P, ec * 512:(ec + 1) * 512],
                                ysb[:])

            npool.release()
            xtbpool.release()
            espool.release()
            vpool.release()
            gpool.release()
        cpool.release()

    nc.finalize()
    return nc


def _get_nc():
    if "nc" not in _CACHE:
        _CACHE["nc"] = _build()
    return _CACHE["nc"]


def _make_in_maps(x, Wq, bq, Wk, bk, Wv, bv, Wfc, bfc):
    x = np.asarray(x, dtype=np.float32)
    Wq = np.asarray(Wq, np.float32); Wk = np.asarray(Wk, np.float32)
    Wv = np.asarray(Wv, np.float32); Wfc = np.asarray(Wfc, np.float32)
    m = np.ascontiguousarray(Wq.T @ Wk)            # q k^T = x m x^T
    n = np.ascontiguousarray(Wv.T @ Wfc.T)         # ((P/denom) @ x) @ n = fc out
    c2v = Wk.T @ np.asarray(bq, np.float32)
    # softmax rows sum to 1, so Wfc@bv + bfc is a constant row of y: fold it
    # (and the residual x) into the xr additive term
    badd = (np.asarray(Wfc, np.float32) @ np.asarray(bv, np.float32)
            + np.asarray(bfc, np.float32))

    in_maps = []
    for core in range(NCORES):
        b, h = core // 2, core % 2
        xtb = np.ascontiguousarray(x[b].T)  # [DIM, S]
        # roll so this core's q-half sits at columns [0, HALF); the k ordering
        # permutes consistently in scores and V, and softmax+sum over k is
        # permutation-invariant, so one SPMD program serves both halves.
        xt = np.ascontiguousarray(np.roll(xtb, -h * HALF, axis=1)) if h else xtb
        xn = np.ascontiguousarray(xt.T)                  # x natural, rolled k-order
        r2 = np.ascontiguousarray(SCALE * (xn @ c2v))    # rolled k-order
        in_maps.append({
            "xt": xt, "xn": xn,
            "xr": np.ascontiguousarray(x[b, h * HALF:(h + 1) * HALF, :] + badd),
            "m": m, "n": n, "r2": r2,
        })
    return in_maps


def kernel(x, Wq, bq, Wk, bk, Wv, bv, Wfc, bfc):
    from concourse.bass_utils import run_bass_kernel_spmd

    nc = _get_nc()
    in_maps = _make_in_maps(x, Wq, bq, Wk, bk, Wv, bv, Wfc, bfc)
    res = run_bass_kernel_spmd(nc, in_maps, core_ids=list(range(NCORES)))
    out = np.empty((B, S, DIM), dtype=np.float32)
    for core in range(NCORES):
        b, h = core // 2, core % 2
        out[b, h * HALF:(h + 1) * HALF, :] = res.results[core]["y"]
    return out



# revision 4
# speedup vs baseline: 1.9844x; 1.9844x over previous
"""AttentionBlock kernel for 8 Trainium2 NeuronCores.

Problem (hardcoded): x [4, 2048, 1024] f32; Wq/Wk/Wv/Wfc [1024, 1024]; biases [1024].
    q = x@Wq.T+bq; k = x@Wk.T+bk; v = x@Wv.T+bv
    out = softmax(q k^T / sqrt(1024)) v;  y = out@Wfc.T+bfc + x

Sharding: core i = (b = i//2, h = i%2). Each core computes the full scores for
its batch element and the attention + fc for its half of the sequence.

Algebra: q k^T = x (Wq^T Wk) x^T = x M x^T, and
    y = softmax(.) x (Wv^T Wfc^T) = (es @ x) @ N / denom, so neither Q/K nor V
is ever materialized. Bias cross-terms fold into a per-k exp bias (r2) and an
additive xr term (residual + Wfc@bv + bfc).

Precision plan (rel err ~1.2e-2 vs 2e-2 budget, verified in numpy sim):
  - score chain in bf16 (same 1 cycle/row PE rate as fp32r, half the DMA):
    G^T = M^T x^T (bf16), S^T = x^T-blocks^T @ G^T (bf16, x resident in SBUF)
  - exp shifted by C=4 (cancels in softmax) so es fits fp8e4m3 (<240)
  - AV side in fp8 with DoubleRow matmuls (256-deep contraction per
    instruction = half the PE instructions):
      denom: ones(=8)-lhsT DR matmul; Z^T = xn-pairs^T @ es-pairs;
      y = zt-pairs^T @ N-pairs, with Z stored fp8*2^-2 and N fed as fp8*2^5;
    the 2^3 net scale cancels against ones=2^3 inside recq = 1/pdr.
  - y normalized by recq as per-partition ACT scale, then + xr (f32), DMA out.
"""

import numpy as np

B, S, DIM = 4, 2048, 1024
P = 128
NCORES = 8
HALF = S // 2          # 1024 q rows per core
DT = DIM // P          # 8 d tiles
SCH = S // 512         # 4 xt column chunks
QC = 512               # attention q-chunk
NQ = HALF // QC        # 2 q chunks
KB = S // P            # 16 k blocks
SCALE = 1.0 / float(np.sqrt(DIM))
CSHIFT = 4.0           # exp bias shift; cancels in softmax
ZSC = 0.25             # Z store scale 2^-2
NSC = 32.0             # N input scale 2^5
ONEV = 8.0             # ones-lhsT value = ZSC*NSC so recq=1/pdr normalizes

_CACHE = {}
TIMING_REPEAT = 21


def _build(repeat=1):
    import concourse.mybir as mybir
    import concourse.tile as tile
    from concourse import bacc

    F32 = mybir.dt.float32
    F32R = mybir.dt.float32r
    BF16 = mybir.dt.bfloat16
    F8 = mybir.dt.float8e4
    DR = mybir.MatmulPerfMode.DoubleRow
    EXP = mybir.ActivationFunctionType.Exp
    IDENT = mybir.ActivationFunctionType.Identity
    ADD = mybir.AluOpType.add

    nc = bacc.Bacc()

    xt_d = nc.dram_tensor("xt", [DIM, S], BF16, kind="ExternalInput")
    xr_d = nc.dram_tensor("xr", [HALF, DIM], F32, kind="ExternalInput")
    m_d = nc.dram_tensor("m", [DIM, DIM], BF16, kind="ExternalInput")  # Wq^T Wk
    n_d = nc.dram_tensor("n", [DIM, DIM], F8, kind="ExternalInput")    # Wv^T Wfc^T * 32
    xn_d = nc.dram_tensor("xn", [S, DIM], F8, kind="ExternalInput")    # x natural, rolled
    r2_d = nc.dram_tensor("r2", [S], F32, kind="ExternalInput")  # scale*x.(Wk^T bq) - C
    y_d = nc.dram_tensor("y", [HALF, DIM], F32, kind="ExternalOutput")

    xt3 = xt_d[:].rearrange("(dt p) s -> p dt s", p=P)      # [128, 8, 2048]
    m3 = m_d[:].rearrange("(dt p) e -> p dt e", p=P)
    n3 = n_d[:].rearrange("(dt p) e -> p dt e", p=P)
    xn3 = xn_d[:].rearrange("(kb p) d -> p kb d", p=P)      # [128, 16, 1024]

    with tile.TileContext(nc, pool_alloc_mode="stack") as tc:
        cpool = tc.alloc_tile_pool(name="const", bufs=1)
        ones2 = cpool.tile([P, 2], F32R)   # HAM-warmup lhsT (even-N fp32r rule)
        # DR denominator lhsT: dual-fp8 ldweights requires the pair dim's
        # byte-stride to be a multiple of 16, so pad the inner dim to 16
        onesk = cpool.tile([P, 2, 16], F8)
        ones_f32 = cpool.tile([P, P], F32)
        nc.vector.memset(ones_f32[:], 1.0)
        nc.vector.tensor_copy(ones2[:], ones_f32[:, 0:2])
        onev_f32 = cpool.tile([P, 2, 16], F32)
        nc.vector.memset(onev_f32[:], ONEV)
        nc.vector.tensor_copy(onesk[:], onev_f32[:])
        # warm the ACT LUTs (first use otherwise pays a ~1.4us cold table load)
        warm = cpool.tile([1, 2], F32)
        nc.scalar.activation(warm[0:1, 0:1], ones_f32[0:1, 0:1], IDENT)
        nc.scalar.activation(warm[0:1, 1:2], ones_f32[0:1, 0:1], EXP)
        # warm the PE HAM clock gate during the initial DMA wait: ~4us of dummy
        # matmuls with no input deps so the real work starts at 2.4GHz
        dwarm = cpool.tile([P, 512], F32R)
        nc.vector.memset(ones_f32[:], 1.0)
        nc.vector.tensor_copy(dwarm[:, 0:P], ones_f32[:])
        with tc.tile_pool(name="pwarm", bufs=1, space="PSUM") as pwp:
            pw = pwp.tile([2, 512], F32)
            for i in range(10):
                nc.tensor.matmul(pw[:], ones2[:], dwarm[:],
                                 start=(i == 0), stop=(i == 9))

        for _rep in range(repeat):
            # -------- Load: xt resident (bf16), plus M; xn/n/r2 in background --
            xpool = tc.alloc_tile_pool(name="xt", bufs=1)
            xt_sb = xpool.tile([P, DT, S], BF16, tag="xt")   # [d_p, dt, k] resident
            gpool = tc.alloc_tile_pool(name="gt", bufs=1)
            gt_sb = gpool.tile([P, DT, HALF], BF16, tag="gt")  # [d_p, dt, q]
            vpool = tc.alloc_tile_pool(name="xn", bufs=1)
            espool = tc.alloc_tile_pool(name="es", bufs=1)
            npool = tc.alloc_tile_pool(name="n", bufs=1)
            xn_sb = vpool.tile([P, KB, DIM], F8, tag="xn")   # [k_p, kb, d]
            r2c = vpool.tile([P, KB], F32, tag="r2c")
            n_sb = npool.tile([P, DT, DIM], F8, tag="n")

            with tc.tile_pool(name="mq", bufs=1) as mqp, \
                 tc.tile_pool(name="pq", bufs=3, space="PSUM") as pqp:
                m_sb = mqp.tile([P, DT, DIM], BF16)
                # phase-G-critical loads first, interleaved across the three
                # DMA queues; S-only xt columns + AV-side tensors follow
                engs = (nc.sync, nc.scalar, nc.gpsimd)
                for dt in range(DT):
                    engs[(2 * dt) % 3].dma_start(m_sb[:, dt, :], m3[:, dt, :])
                    engs[(2 * dt + 1) % 3].dma_start(
                        xt_sb[:, dt, 0:512], xt3[:, dt, 0:512])
                for dt in range(DT):
                    engs[dt % 3].dma_start(
                        xt_sb[:, dt, 512:1024], xt3[:, dt, 512:1024])
                nc.sync.dma_start(xt_sb[:, :, 1024:2048], xt3[:, :, 1024:2048])
                nc.scalar.dma_start(r2c[:], r2_d[:].rearrange("(t p) -> p t", p=P))
                for kb in range(KB):
                    nc.gpsimd.dma_start(xn_sb[:, kb, :], xn3[:, kb, :])
                nc.gpsimd.dma_start(n_sb[:], n3[:])

                # ---- Phase G: G^T = M-blocks^T @ xT-half (bf16, resident) ----
                for qch in range(HALF // 512):
                    for dtile in range(DT):
                        ps = pqp.tile([P, 512], F32, tag="pq")
                        for dt in range(DT):
                            nc.tensor.matmul(
                                ps[:], m_sb[:, dt, dtile * P:(dtile + 1) * P],
                                xt_sb[:, dt, qch * 512:(qch + 1) * 512],
                                start=(dt == 0), stop=(dt == DT - 1))
                        nc.scalar.activation(
                            gt_sb[:, dtile, qch * 512:(qch + 1) * 512], ps[:], IDENT)

            # ------- Phase A: attention -> y directly (per q-chunk of 512) ------
            with tc.tile_pool(name="zt", bufs=1) as ztp, \
                 tc.tile_pool(name="rec", bufs=2) as recp, \
                 tc.tile_pool(name="xrt", bufs=2) as xrp, \
                 tc.tile_pool(name="ysb", bufs=2) as ysp, \
                 tc.tile_pool(name="ps_s", bufs=2, space="PSUM") as psp, \
                 tc.tile_pool(name="ps_z", bufs=4, space="PSUM") as pzp, \
                 tc.tile_pool(name="ps_d", bufs=1, space="PSUM") as pdp:
                for qc in range(NQ):
                    q0 = qc * QC
                    es = espool.tile([P, KB, QC], F8, tag="es")  # exp [k_p, kb, q]
                    pdr = pdp.tile([1, QC], F32, tag="ps_d")
                    for kb in range(KB):
                        ps = psp.tile([P, QC], F32, tag="ps_s")
                        for dt in range(DT):
                            nc.tensor.matmul(
                                ps[:], xt_sb[:, dt, kb * P:(kb + 1) * P],
                                gt_sb[:, dt, q0:q0 + QC],
                                start=(dt == 0), stop=(dt == DT - 1))
                        nc.scalar.activation(es[:, kb, :], ps[:], EXP,
                                             bias=r2c[:, kb:kb + 1], scale=SCALE)
                    # denominator after the S loop: interleaving it would stall
                    # PE on each odd kb's exp (ACT latency) before the next
                    # S-block; here only the last exp gates it, once
                    for kp in range(KB // 2):
                        nc.tensor.matmul(pdr[:], onesk[:, :, 0:1],
                                         es[:, 2 * kp:2 * kp + 2, :],
                                         start=(kp == 0), stop=(kp == KB // 2 - 1),
                                         perf_mode=DR)
                    # reciprocal row -> per-q columns via 4 partition-column DMAs
                    recd = recp.tile([1, QC], F32, tag="recd")
                    nc.vector.reciprocal(recd[:], pdr[:])
                    recq = recp.tile([P, QC // P], F32, tag="recq")
                    for qb in range(QC // P):
                        nc.sync.dma_start(recq[:, qb:qb + 1],
                                          recd[0:1, qb * P:(qb + 1) * P])
                    # Z~^T[d, q] = sum_kb xn-pair^T @ es-pair (DR), 4 banks/pass
                    zt = ztp.tile([P, DT, QC], F8, tag="zt")  # [d_p, dt, q]
                    for hf in range(2):
                        pz = [pzp.tile([P, QC], F32, tag="ps_z", name=f"pz{hf}_{i}")
                              for i in range(4)]
                        for kp in range(KB // 2):
                            for i in range(4):
                                dtile = hf * 4 + i
                                nc.tensor.matmul(
                                    pz[i][:],
                                    xn_sb[:, 2 * kp:2 * kp + 2,
                                          dtile * P:(dtile + 1) * P],
                                    es[:, 2 * kp:2 * kp + 2, :],
                                    start=(kp == 0), stop=(kp == KB // 2 - 1),
                                    perf_mode=DR)
                        for i in range(4):
                            nc.scalar.activation(zt[:, hf * 4 + i, :], pz[i][:],
                                                 IDENT, scale=ZSC)
                    # y = zt-pairs^T @ N-pairs (DR), scaled by 1/denom; xr biases
                    for qb in range(QC // P):
                        q_t = qc * (QC // P) + qb
                        xrt = xrp.tile([P, DIM], F32, tag="xrt")
                        nc.scalar.dma_start(xrt[:], xr_d[q_t * P:(q_t + 1) * P, :])
                        for ec in range(2):
                            py = pzp.tile([P, 512], F32, tag="ps_z",
                                          name=f"py{q_t}_{ec}")
                            for dp in range(DT // 2):
                                nc.tensor.matmul(
                                    py[:],
                                    zt[:, 2 * dp:2 * dp + 2, qb * P:(qb + 1) * P],
                                    n_sb[:, 2 * dp:2 * dp + 2,
                                         ec * 512:(ec + 1) * 512],
                                    start=(dp == 0), stop=(dp == DT // 2 - 1),
                                    perf_mode=DR)
                            ysb = ysp.tile([P, 512], F32, tag="ysb")
                            nc.scalar.activation(ysb[:], py[:], IDENT,
                                                 scale=recq[:, qb:qb + 1])
                            nc.vector.tensor_tensor(
                                ysb[:], ysb[:], xrt[:, ec * 512:(ec + 1) * 512], ADD)
                            nc.gpsimd.dma_start(
                                y_d[q_t * P:(q_t + 1) * P, ec * 512:(ec + 1) * 512],
                                ysb[:])

            npool.release()
            espool.release()
            vpool.release()
            gpool.release()
            xpool.release()
        cpool.release()

    nc.finalize()
    return nc


def _get_nc():
    if "nc" not in _CACHE:
        _CACHE["nc"] = _build()
    return _CACHE["nc"]


def _make_in_maps(x, Wq, bq, Wk, bk, Wv, bv, Wfc, bfc):
    import ml_dtypes
    BF = ml_dtypes.bfloat16
    F8 = ml_dtypes.float8_e4m3

    x = np.asarray(x, dtype=np.float32)
    Wq = np.asarray(Wq, np.float32); Wk = np.asarray(Wk, np.float32)
    Wv = np.asarray(Wv, np.float32); Wfc = np.asarray(Wfc, np.float32)
    m = np.ascontiguousarray(Wq.T @ Wk).astype(BF)       # q k^T = x m x^T
    n = np.ascontiguousarray((Wv.T @ Wfc.T) * NSC).astype(F8)
    c2v = Wk.T @ np.asarray(bq, np.float32)
    # softmax rows sum to 1, so Wfc@bv + bfc is a constant row of y: fold it
    # (and the residual x) into the xr additive term
    badd = (np.asarray(Wfc, np.float32) @ np.asarray(bv, np.float32)
            + np.asarray(bfc, np.float32))

    in_maps = []
    for core in range(NCORES):
        b, h = core // 2, core % 2
        xtb = np.ascontiguousarray(x[b].T)  # [DIM, S]
        # roll so this core's q-half sits at columns [0, HALF); the k ordering
        # permutes consistently in scores and V, and softmax+sum over k is
        # permutation-invariant, so one SPMD program serves both halves.
        xt = np.ascontiguousarray(np.roll(xtb, -h * HALF, axis=1)) if h else xtb
        xnf = np.ascontiguousarray(xt.T)                 # x natural, rolled k-order
        r2 = np.ascontiguousarray(SCALE * (xnf @ c2v) - CSHIFT)
        in_maps.append({
            "xt": np.ascontiguousarray(xt.astype(BF)),
            "xn": np.ascontiguousarray(xnf.astype(F8)),
            "xr": np.ascontiguousarray(x[b, h * HALF:(h + 1) * HALF, :] + badd),
            "m": m, "n": n, "r2": r2,
        })
    return in_maps


def kernel(x, Wq, bq, Wk, bk, Wv, bv, Wfc, bfc):
    from concourse.bass_utils import run_bass_kernel_spmd

    nc = _get_nc()
    in_maps = _make_in_maps(x, Wq, bq, Wk, bk, Wv, bv, Wfc, bfc)
    res = run_bass_kernel_spmd(nc, in_maps, core_ids=list(range(NCORES)))
    out = np.empty((B, S, DIM), dtype=np.float32)
    for core in range(NCORES):
        b, h = core // 2, core % 2
        out[b, h * HALF:(h + 1) * HALF, :] = res.results[core]["y"]
    return out


# revision 22
# speedup vs baseline: 2.9173x; 1.4701x over previous
"""AttentionBlock kernel for 8 Trainium2 NeuronCores.

Problem (hardcoded): x [4, 2048, 1024] f32; Wq/Wk/Wv/Wfc [1024, 1024]; biases [1024].
    q = x@Wq.T+bq; k = x@Wk.T+bk; v = x@Wv.T+bv
    out = softmax(q k^T / sqrt(1024)) v;  y = out@Wfc.T+bfc + x

Sharding: core i = (b = i//2, h = i%2). Each core computes the full scores for
its batch element and the attention + fc for its half of the sequence.

Algebra: q k^T = x (Wq^T Wk) x^T = x M x^T, and
    y = softmax(.) x (Wv^T Wfc^T) = (es @ x) @ N / denom, so neither Q/K nor V
is ever materialized. Bias cross-terms fold into a per-k exp bias (r2) and an
additive xr term (residual + Wfc@bv + bfc).

Precision plan (rel err ~1.2e-2 vs 2e-2 budget, verified in numpy sim and HW):
  - score chain in bf16 (same 1 cycle/row PE rate as fp32r, half the DMA):
    G^T = M^T x^T (bf16), S^T = x^T-blocks^T @ G^T (bf16, x resident in SBUF)
  - exp shifted by C=4 (cancels in softmax) so es fits TRN fp8e4m3 (max 240)
  - AV side in fp8 with DoubleRow matmuls (256-deep contraction per
    instruction ~ 160ns vs 196ns for a 128-deep bf16 matmul, measured):
      denom: ones(=8)-lhsT DR matmul; Z^T = xn-pairs^T @ es-pairs;
      y = zt-pairs^T @ N-pairs, with Z stored fp8*2^-2 and N fed as fp8*2^5;
    the 2^3 net scale cancels against ones=2^3 inside recq = 1/pdr.
  - y normalized by recq as per-partition ACT scale, then + xr (f32), DMA out.

Pipeline notes (measured on HW, slope of repeat-body wall time):
  - G-critical loads (m, xt[:, :512]) go on sync+scalar only: gpsimd drains the
    previous body's y-output DMAs at the boundary.
  - denominator DR matmuls run between the two Z half-passes (never stall PE
    on the last exp); dual-fp8 ldweights needs pair-stride%16==0 (onesk pad).
  - y outputs alternate gpsimd/sync queues; ysb bufs=6 so the DMA drain never
    backs up PE; xr tiles prefetch during Z.
  - fp8 hi/lo splits for the score chain do NOT pay: 3x instruction count at
    ~1.2x per-instruction DR advantage is a net loss; scores stay bf16.
"""

import numpy as np

B, S, DIM = 4, 2048, 1024
P = 128
NCORES = 8
HALF = S // 2          # 1024 q rows per core
DT = DIM // P          # 8 d tiles
QC = 512               # attention q-chunk
NQ = HALF // QC        # 2 q chunks
KB = S // P            # 16 k blocks
SCALE = 1.0 / float(np.sqrt(DIM))
CSHIFT = 4.0           # exp bias shift; cancels in softmax
ZSC = 0.25             # Z store scale 2^-2
NSC = 32.0             # N input scale 2^5
ONEV = 8.0             # ones-lhsT value = ZSC*NSC so recq=1/pdr normalizes

_CACHE = {}
TIMING_REPEAT = 21
# Timing-ablation hook: bench_abl.py sets this to "S2"/"Z2"/"E2" before _build
# to duplicate one instruction class and measure its in-situ marginal cost.
ABLATE = None


def _build(repeat=1):
    import concourse.mybir as mybir
    import concourse.tile as tile
    from concourse import bacc

    F32 = mybir.dt.float32
    F32R = mybir.dt.float32r
    BF16 = mybir.dt.bfloat16
    F8 = mybir.dt.float8e4
    DR = mybir.MatmulPerfMode.DoubleRow
    EXP = mybir.ActivationFunctionType.Exp
    IDENT = mybir.ActivationFunctionType.Identity
    ADD = mybir.AluOpType.add

    nc = bacc.Bacc()

    xt_d = nc.dram_tensor("xt", [DIM, S], BF16, kind="ExternalInput")
    xr_d = nc.dram_tensor("xr", [HALF, DIM], F32, kind="ExternalInput")
    m_d = nc.dram_tensor("m", [DIM, DIM], BF16, kind="ExternalInput")  # Wq^T Wk
    n_d = nc.dram_tensor("n", [DIM, DIM], F8, kind="ExternalInput")    # Wv^T Wfc^T * 32
    xn_d = nc.dram_tensor("xn", [S, DIM], F8, kind="ExternalInput")    # x natural, rolled
    r2_d = nc.dram_tensor("r2", [S], F32, kind="ExternalInput")  # scale*x.(Wk^T bq) - C
    y_d = nc.dram_tensor("y", [HALF, DIM], F32, kind="ExternalOutput")

    xt3 = xt_d[:].rearrange("(dt p) s -> p dt s", p=P)      # [128, 8, 2048]
    m3 = m_d[:].rearrange("(dt p) e -> p dt e", p=P)
    n3 = n_d[:].rearrange("(dt p) e -> p dt e", p=P)
    xn3 = xn_d[:].rearrange("(kb p) d -> p kb d", p=P)      # [128, 16, 1024]

    with tile.TileContext(nc, pool_alloc_mode="stack") as tc:
        cpool = tc.alloc_tile_pool(name="const", bufs=1)
        ones2 = cpool.tile([P, 2], F32R)   # HAM-warmup lhsT (even-N fp32r rule)
        # DR denominator lhsT: dual-fp8 ldweights requires the pair dim's
        # byte-stride to be a multiple of 16, so pad the inner dim to 16
        onesk = cpool.tile([P, 2, 16], F8)
        ones_f32 = cpool.tile([P, P], F32)
        nc.vector.memset(ones_f32[:], 1.0)
        nc.vector.tensor_copy(ones2[:], ones_f32[:, 0:2])
        onev_f32 = cpool.tile([P, 2, 16], F32)
        nc.vector.memset(onev_f32[:], ONEV)
        nc.vector.tensor_copy(onesk[:], onev_f32[:])
        # warm the ACT LUTs (first use otherwise pays a ~1.4us cold table load)
        warm = cpool.tile([1, 2], F32)
        nc.scalar.activation(warm[0:1, 0:1], ones_f32[0:1, 0:1], IDENT)
        nc.scalar.activation(warm[0:1, 1:2], ones_f32[0:1, 0:1], EXP)
        # warm the PE HAM clock gate during the initial DMA wait: ~4us of dummy
        # matmuls with no input deps so the real work starts at 2.4GHz
        dwarm = cpool.tile([P, 512], F32R)
        nc.vector.memset(ones_f32[:], 1.0)
        nc.vector.tensor_copy(dwarm[:, 0:P], ones_f32[:])
        with tc.tile_pool(name="pwarm", bufs=1, space="PSUM") as pwp:
            pw = pwp.tile([2, 512], F32)
            for i in range(10):
                nc.tensor.matmul(pw[:], ones2[:], dwarm[:],
                                 start=(i == 0), stop=(i == 9))

        for _rep in range(repeat):
            # -------- Load: xt resident (bf16), plus M; xn/n/r2 in background --
            xpool = tc.alloc_tile_pool(name="xt", bufs=1)
            xt_sb = xpool.tile([P, DT, S], BF16, tag="xt")   # [d_p, dt, k] resident
            gpool = tc.alloc_tile_pool(name="gt", bufs=1)
            gt_sb = gpool.tile([P, DT, HALF], BF16, tag="gt")  # [d_p, dt, q]
            vpool = tc.alloc_tile_pool(name="xn", bufs=1)
            espool = tc.alloc_tile_pool(name="es", bufs=1)
            npool = tc.alloc_tile_pool(name="n", bufs=1)
            xn_sb = vpool.tile([P, KB, DIM], F8, tag="xn")   # [k_p, kb, d]
            r2c = vpool.tile([P, KB], F32, tag="r2c")
            n_sb = npool.tile([P, DT, DIM], F8, tag="n")

            with tc.tile_pool(name="mq", bufs=1) as mqp, \
                 tc.tile_pool(name="pq", bufs=3, space="PSUM") as pqp:
                m_sb = mqp.tile([P, DT, DIM], BF16)
                # Phase-G-critical loads (m + xt cols 0:512) split over sync and
                # scalar only: gpsimd still drains the previous body's y-output
                # DMAs at the boundary, so nothing G-critical goes there.
                for dt in range(0, DT, 2):
                    nc.sync.dma_start(m_sb[:, dt, :], m3[:, dt, :])
                    nc.scalar.dma_start(m_sb[:, dt + 1, :], m3[:, dt + 1, :])
                for dt in range(DT):
                    (nc.scalar if dt % 2 == 0 else nc.sync).dma_start(
                        xt_sb[:, dt, 0:512], xt3[:, dt, 0:512])
                for dt in range(DT):
                    nc.scalar.dma_start(
                        xt_sb[:, dt, 512:1024], xt3[:, dt, 512:1024])
                for dt in range(DT):
                    nc.sync.dma_start(xt_sb[:, dt, 1024:2048],
                                      xt3[:, dt, 1024:2048])
                nc.scalar.dma_start(r2c[:], r2_d[:].rearrange("(t p) -> p t", p=P))
                for kb in range(KB):
                    nc.gpsimd.dma_start(xn_sb[:, kb, :], xn3[:, kb, :])
                nc.gpsimd.dma_start(n_sb[:], n3[:])

                # ---- Phase G: G^T = M-blocks^T @ xT-half (bf16, resident) ----
                for qch in range(HALF // 512):
                    for dtile in range(DT):
                        ps = pqp.tile([P, 512], F32, tag="pq")
                        for dt in range(DT):
                            nc.tensor.matmul(
                                ps[:], m_sb[:, dt, dtile * P:(dtile + 1) * P],
                                xt_sb[:, dt, qch * 512:(qch + 1) * 512],
                                start=(dt == 0), stop=(dt == DT - 1))
                        nc.scalar.activation(
                            gt_sb[:, dtile, qch * 512:(qch + 1) * 512], ps[:], IDENT)

            # ------- Phase A: attention -> y directly (per q-chunk of 512) ------
            with tc.tile_pool(name="zt", bufs=1) as ztp, \
                 tc.tile_pool(name="rec", bufs=2) as recp, \
                 tc.tile_pool(name="xrt", bufs=4) as xrp, \
                 tc.tile_pool(name="ysb", bufs=6) as ysp, \
                 tc.tile_pool(name="ps_s", bufs=3, space="PSUM") as psp, \
                 tc.tile_pool(name="ps_z", bufs=4, space="PSUM") as pzp, \
                 tc.tile_pool(name="ps_d", bufs=1, space="PSUM") as pdp:
                for qc in range(NQ):
                    q0 = qc * QC
                    es = espool.tile([P, KB, QC], F8, tag="es")  # exp [k_p, kb, q]
                    pdr = pdp.tile([1, QC], F32, tag="ps_d")
                    srep = 2 if ABLATE == "S2" else 1
                    for kb in range(KB):
                        ps = psp.tile([P, QC], F32, tag="ps_s")
                        for sr in range(srep):
                            for dt in range(DT):
                                nc.tensor.matmul(
                                    ps[:], xt_sb[:, dt, kb * P:(kb + 1) * P],
                                    gt_sb[:, dt, q0:q0 + QC],
                                    start=(sr == 0 and dt == 0),
                                    stop=(sr == srep - 1 and dt == DT - 1))
                        for _er in range(2 if ABLATE == "E2" else 1):
                            nc.scalar.activation(es[:, kb, :], ps[:], EXP,
                                                 bias=r2c[:, kb:kb + 1], scale=SCALE)
                    # prefetch this q-chunk's residual tiles during the Z phase
                    xrts = []
                    for qb in range(QC // P):
                        q_t = qc * (QC // P) + qb
                        xrt = xrp.tile([P, DIM], F32, tag="xrt")
                        nc.scalar.dma_start(xrt[:], xr_d[q_t * P:(q_t + 1) * P, :])
                        xrts.append(xrt)
                    # Z~^T[d, q] = sum_kb xn-pair^T @ es-pair (DR), 4 banks/pass.
                    # The denominator matmuls run between the two half-passes so
                    # they never stall PE on the last exp's ACT latency, and the
                    # reciprocal->recq chain still completes well before y reads.
                    zt = ztp.tile([P, DT, QC], F8, tag="zt")  # [d_p, dt, q]
                    zrep = 2 if ABLATE == "Z2" else 1
                    recd = recp.tile([1, QC], F32, tag="recd")
                    recq = recp.tile([P, QC // P], F32, tag="recq")
                    for hf in range(2):
                        pz = [pzp.tile([P, QC], F32, tag="ps_z", name=f"pz{hf}_{i}")
                              for i in range(4)]
                        for zr in range(zrep):
                            for kp in range(KB // 2):
                                for i in range(4):
                                    dtile = hf * 4 + i
                                    nc.tensor.matmul(
                                        pz[i][:],
                                        xn_sb[:, 2 * kp:2 * kp + 2,
                                              dtile * P:(dtile + 1) * P],
                                        es[:, 2 * kp:2 * kp + 2, :],
                                        start=(zr == 0 and kp == 0),
                                        stop=(zr == zrep - 1 and kp == KB // 2 - 1),
                                        perf_mode=DR)
                        for i in range(4):
                            nc.scalar.activation(zt[:, hf * 4 + i, :], pz[i][:],
                                                 IDENT, scale=ZSC)
                        if hf == 0:
                            for kp in range(KB // 2):
                                nc.tensor.matmul(pdr[:], onesk[:, :, 0:1],
                                                 es[:, 2 * kp:2 * kp + 2, :],
                                                 start=(kp == 0),
                                                 stop=(kp == KB // 2 - 1),
                                                 perf_mode=DR)
                            nc.vector.reciprocal(recd[:], pdr[:])
                            for qb in range(QC // P):
                                nc.sync.dma_start(recq[:, qb:qb + 1],
                                                  recd[0:1, qb * P:(qb + 1) * P])
                    # y = zt-pairs^T @ N-pairs (DR), scaled by 1/denom; xr biases.
                    # Output DMAs alternate gpsimd/sync so the drain rate
                    # (~12us/qc on one queue) never backs up the ysb pool.
                    for qb in range(QC // P):
                        q_t = qc * (QC // P) + qb
                        xrt = xrts[qb]
                        for ec in range(2):
                            py = pzp.tile([P, 512], F32, tag="ps_z",
                                          name=f"py{q_t}_{ec}")
                            for dp in range(DT // 2):
                                nc.tensor.matmul(
                                    py[:],
                                    zt[:, 2 * dp:2 * dp + 2, qb * P:(qb + 1) * P],
                                    n_sb[:, 2 * dp:2 * dp + 2,
                                         ec * 512:(ec + 1) * 512],
                                    start=(dp == 0), stop=(dp == DT // 2 - 1),
                                    perf_mode=DR)
                            ysb = ysp.tile([P, 512], F32, tag="ysb")
                            nc.scalar.activation(ysb[:], py[:], IDENT,
                                                 scale=recq[:, qb:qb + 1])
                            nc.vector.tensor_tensor(
                                ysb[:], ysb[:], xrt[:, ec * 512:(ec + 1) * 512], ADD)
                            (nc.gpsimd if ec == 0 else nc.sync).dma_start(
                                y_d[q_t * P:(q_t + 1) * P, ec * 512:(ec + 1) * 512],
                                ysb[:])

            npool.release()
            espool.release()
            vpool.release()
            gpool.release()
            xpool.release()
        cpool.release()

    nc.finalize()
    return nc


def _get_nc():
    if "nc" not in _CACHE:
        _CACHE["nc"] = _build()
    return _CACHE["nc"]


def _make_in_maps(x, Wq, bq, Wk, bk, Wv, bv, Wfc, bfc):
    import ml_dtypes
    BF = ml_dtypes.bfloat16
    F8 = ml_dtypes.float8_e4m3

    x = np.asarray(x, dtype=np.float32)
    Wq = np.asarray(Wq, np.float32); Wk = np.asarray(Wk, np.float32)
    Wv = np.asarray(Wv, np.float32); Wfc = np.asarray(Wfc, np.float32)
    m = np.ascontiguousarray(Wq.T @ Wk).astype(BF)       # q k^T = x m x^T
    n = np.ascontiguousarray((Wv.T @ Wfc.T) * NSC).astype(F8)
    c2v = Wk.T @ np.asarray(bq, np.float32)
    # softmax rows sum to 1, so Wfc@bv + bfc is a constant row of y: fold it
    # (and the residual x) into the xr additive term
    badd = (np.asarray(Wfc, np.float32) @ np.asarray(bv, np.float32)
            + np.asarray(bfc, np.float32))

    in_maps = []
    for core in range(NCORES):
        b, h = core // 2, core % 2
        xtb = np.ascontiguousarray(x[b].T)  # [DIM, S]
        # roll so this core's q-half sits at columns [0, HALF); the k ordering
        # permutes consistently in scores and V, and softmax+sum over k is
        # permutation-invariant, so one SPMD program serves both halves.
        xt = np.ascontiguousarray(np.roll(xtb, -h * HALF, axis=1)) if h else xtb
        xnf = np.ascontiguousarray(xt.T)                 # x natural, rolled k-order
        r2 = np.ascontiguousarray(SCALE * (xnf @ c2v) - CSHIFT)
        in_maps.append({
            "xt": np.ascontiguousarray(xt.astype(BF)),
            "xn": np.ascontiguousarray(xnf.astype(F8)),
            "xr": np.ascontiguousarray(x[b, h * HALF:(h + 1) * HALF, :] + badd),
            "m": m, "n": n, "r2": r2,
        })
    return in_maps


def kernel(x, Wq, bq, Wk, bk, Wv, bv, Wfc, bfc):
    from concourse.bass_utils import run_bass_kernel_spmd

    nc = _get_nc()
    in_maps = _make_in_maps(x, Wq, bq, Wk, bk, Wv, bv, Wfc, bfc)
    res = run_bass_kernel_spmd(nc, in_maps, core_ids=list(range(NCORES)))
    out = np.empty((B, S, DIM), dtype=np.float32)
    for core in range(NCORES):
        b, h = core // 2, core % 2
        out[b, h * HALF:(h + 1) * HALF, :] = res.results[core]["y"]
    return out
